# revision 5
# baseline (speedup 1.0000x reference)
"""GCN layer (dense projection + sparse neighbor aggregation) on 8 Trainium2
NeuronCores via Bass/Tile.

Strategy: shard nodes (and their incident edges, grouped by destination row)
across the 8 cores; replicate W/b; AllGather the projected node features in 7
quarter-collectives merged into 4 int16-addressable sub-tables; per 128-row
output block, bulk-gather the needed source rows with DMAGatherAnt, fold
edge_val into the fp32->fp16 cast on the scalar engine, and segment-sum via an
assignment-matrix matmul accumulated in PSUM (bias folded in as an extra
rank-128 matmul; padded gather slots are killed by rowloc=-1).

Phase 2 runs bucket-major with an SBUF accumulator per output block, so
bucket b's gathers start as soon as quarters 2b/2b+1 have AllGathered —
collective triggers are emitted interleaved with the gather stream to keep
the GpSimd queue from stalling.  Chunk capacities are per-(block, bucket)
maxima over cores (program stays SPMD-uniform, ~13% fewer descriptors than
per-bucket uniform caps).
"""

import sys

if "/opt/trn_rl_repo" not in sys.path:
    sys.path.insert(0, "/opt/trn_rl_repo")

import numpy as np

import concourse.bass as bass
import concourse.mybir as mybir
import concourse.tile as tile
from concourse import bacc
from concourse.bass_utils import run_bass_kernel_spmd

N_NODES = 100000
N_EDGES = 1600000
IN_FT = 256
OUT_FT = 64
NCORES = 8
NS = N_NODES // NCORES          # 12500 nodes per core
NB = (NS + 127) // 128          # 98 row blocks per core
NSP = NB * 128                  # 12544 padded nodes per core
GB = 7                          # row blocks per group (98 = 14 * 7)
NGROUPS = NB // GB              # 14
# buckets = int16-addressable sub-tables, one AllGather each; the first is a
# single group so gathers start as soon as possible
BGROUPS = [[0], [1, 2, 3], [4, 5, 6, 7], [8, 9, 10, 11], [12, 13]]
NBUCKET = len(BGROUPS)
BUCKET_OF_G = [b for b, gs in enumerate(BGROUPS) for _ in gs]
BBLK0 = [gs[0] * GB for gs in BGROUPS]          # first block per bucket
BNBLK = [len(gs) * GB for gs in BGROUPS]        # blocks per bucket
BROWS = [NCORES * 128 * n for n in BNBLK]       # rows per sub-table

F32 = mybir.dt.float32
F16 = mybir.dt.float16
I32 = mybir.dt.int32
I16 = mybir.dt.int16

MAXCH = 8                       # 1024 indices = HW cap per dma_gather
NQ = 4                          # SWDGE queues (set to 1 for CoreSim runs)


def _chunk_layout(cap4):
    """Stream order: bucket-major, then group, then block.

    Returns (start[NB][4] chunk offsets, tot_chunks, region[(bkt, g)] =
    (chunk_lo, chunk_hi))."""
    start = np.zeros((NB, NBUCKET), np.int64)
    region = {}
    off = 0
    for bkt in range(NBUCKET):
        for g in range(NGROUPS):
            lo = off
            for j in range(GB):
                blk = g * GB + j
                start[blk, bkt] = off
                off += int(cap4[blk][bkt])
            region[(bkt, g)] = (lo, off)
    return start, int(off), region


def build_program(cap4):
    """One SPMD Bass program; all 8 cores run it on their own shards."""
    cap4 = [list(map(int, row)) for row in cap4]
    start, tot, region = _chunk_layout(cap4)
    gcols = tot * 8

    nc = bacc.Bacc("TRN2", target_bir_lowering=False, debug=False,
                   num_devices=NCORES, num_swdge_queues=NQ)

    seqT = nc.dram_tensor("seqT", [2, 128, NSP], F32, kind="ExternalInput")
    gidx = nc.dram_tensor("gidx", [128, gcols], I16, kind="ExternalInput")
    val = nc.dram_tensor("val", [128, tot], F16, kind="ExternalInput")
    rl = nc.dram_tensor("rl", [128, tot], F16, kind="ExternalInput")
    w_in = nc.dram_tensor("w", [128, 2, OUT_FT], F32, kind="ExternalInput")
    bias_in = nc.dram_tensor("biasb", [128, OUT_FT], F16,
                             kind="ExternalInput")
    # partition-major layouts: [p, block, feature]; host un-permutes
    sf_out = nc.dram_tensor("sf", [128, NB, OUT_FT], F32,
                            kind="ExternalOutput")
    agg_out = nc.dram_tensor("agg", [128, NB, OUT_FT], F32,
                             kind="ExternalOutput")
    ccin = [nc.dram_tensor(f"ccin{b}", [128, BNBLK[b], OUT_FT], F32)
            for b in range(NBUCKET)]
    xt = [nc.dram_tensor(f"xt{b}", [BROWS[b], OUT_FT], F32,
                         addr_space="Shared") for b in range(NBUCKET)]

    groups = [list(range(NCORES))]
    qrr = [0]

    with tile.TileContext(nc) as tc:
        with (
            tc.tile_pool(name="const", bufs=1) as cpool,
            tc.tile_pool(name="psum", bufs=2, space="PSUM") as psum_pool,
        ):
            w_sb = cpool.tile([128, 2, OUT_FT], F32)
            nc.sync.dma_start(out=w_sb[:], in_=w_in[:])
            bias_sb = cpool.tile([128, OUT_FT], F16)
            nc.sync.dma_start(out=bias_sb[:], in_=bias_in[:])
            ones_sb = cpool.tile([128, 128], F16)
            nc.gpsimd.memset(ones_sb[:], 1.0)
            iota_i = cpool.tile([128, 128], I32)
            nc.gpsimd.iota(iota_i[:], pattern=[[1, 128]], base=0,
                           channel_multiplier=0)
            iota_f = cpool.tile([128, 128], F16)
            nc.vector.tensor_copy(out=iota_f[:], in_=iota_i[:])

            # ---- phase 1: x = seq @ W (fp32); quarter ccin staged ----
            with (
                tc.tile_pool(name="seqpool", bufs=1) as seqpool,
                tc.tile_pool(name="p1work", bufs=3) as p1work,
            ):
                seqT_sb = seqpool.tile([128, 2, NSP], F32)
                PAN = 2 * GB * 128          # 2 groups per panel
                for kc in range(2):
                    for p0 in range(0, NSP, PAN):
                        p1 = min(NSP, p0 + PAN)
                        nc.sync.dma_start(out=seqT_sb[:, kc, p0:p1],
                                          in_=seqT[kc, :, p0:p1])

                for g in range(NGROUPS):
                    x_sb = p1work.tile([128, GB, OUT_FT], F32, tag="x_sb")
                    for j in range(GB):
                        nb = g * GB + j
                        px = psum_pool.tile([128, OUT_FT], F32, tag="px")
                        for kc in range(2):
                            nc.tensor.matmul(
                                px[:],
                                seqT_sb[:, kc, nb * 128:(nb + 1) * 128],
                                w_sb[:, kc, :],
                                start=(kc == 0),
                                stop=(kc == 1),
                            )
                        nc.vector.tensor_copy(out=x_sb[:, j, :], in_=px[:])
                    nc.sync.dma_start(
                        out=sf_out[:, g * GB:(g + 1) * GB, :], in_=x_sb[:])
                    b_ = BUCKET_OF_G[g]
                    g0 = (g - BGROUPS[b_][0]) * GB
                    nc.sync.dma_start(
                        out=ccin[b_][:, g0:g0 + GB, :], in_=x_sb[:])

            # ---- phase 2: bucket-major gather + scale + segsum matmul ----
            with (
                tc.tile_pool(name="accpool", bufs=1) as accpool,
                tc.tile_pool(name="edgemeta", bufs=1) as mpool,
                tc.tile_pool(name="p2work", bufs=4) as p2,
            ):
                acc = accpool.tile([128, NB, OUT_FT], F32)
                gidx_sb = mpool.tile([128, gcols], I16)
                nc.sync.dma_start(out=gidx_sb[:], in_=gidx[:])
                val_sb = mpool.tile([128, tot], F16)
                nc.sync.dma_start(out=val_sb[:], in_=val[:])
                rl_sb = mpool.tile([128, tot], F16)
                nc.sync.dma_start(out=rl_sb[:], in_=rl[:])

                for bkt in range(NBUCKET):
                    nc.gpsimd.collective_compute(
                        "AllGather",
                        mybir.AluOpType.bypass,
                        replica_groups=groups,
                        ins=[ccin[bkt][:]],
                        outs=[xt[bkt][:]],
                    )
                    for g in range(NGROUPS):
                        lo, hi = region[(bkt, g)]
                        nreg = hi - lo
                        # one region-wide gather target + fp16 msg buffer so
                        # the gather stream runs far ahead of consumers
                        xg = p2.tile([128, nreg, OUT_FT], F32, tag="xg")
                        c0 = lo
                        while c0 < hi:
                            ln = min(MAXCH, hi - c0)
                            nc.gpsimd.dma_gather(
                                out_ap=xg[:, c0 - lo:c0 - lo + ln, :],
                                in_ap=xt[bkt][:],
                                idxs_ap=gidx_sb[:, c0 * 8:(c0 + ln) * 8],
                                num_idxs=ln * 128,
                                num_idxs_reg=ln * 128,
                                elem_size=OUT_FT,
                                queue_num=qrr[0] % NQ,
                            )
                            qrr[0] += 1
                            c0 += ln
                        msg = p2.tile([128, nreg, OUT_FT], F16, tag="msg")
                        nc.vector.tensor_tensor(
                            out=msg[:],
                            in0=xg[:],
                            in1=val_sb[:, lo:hi].unsqueeze(
                                2).broadcast_to([128, nreg, OUT_FT]),
                            op=mybir.AluOpType.mult,
                        )
                        msg_of = {ci: (msg, ci - lo) for ci in range(lo, hi)}
                        o_sb = None
                        if bkt == NBUCKET - 1:
                            o_sb = p2.tile([128, GB, OUT_FT], F32, tag="o_sb")
                        for j in range(GB):
                            blk = g * GB + j
                            nch = cap4[blk][bkt]
                            s = int(start[blk, bkt])
                            a_sb = p2.tile([128, nch * 128], F16, tag="a_sb")
                            nc.vector.tensor_tensor(
                                out=a_sb[:].rearrange("p (c q) -> p c q",
                                                      q=128),
                                in0=rl_sb[:, s:s + nch].unsqueeze(
                                    2).broadcast_to([128, nch, 128]),
                                in1=iota_f[:].unsqueeze(1).broadcast_to(
                                    [128, nch, 128]),
                                op=mybir.AluOpType.is_equal,
                            )
                            po = psum_pool.tile([128, OUT_FT], F32, tag="po")
                            if bkt == 0:
                                nc.tensor.matmul(po[:], ones_sb[:],
                                                 bias_sb[:],
                                                 start=True, stop=False)
                            for cc in range(nch):
                                mt, mi = msg_of[s + cc]
                                nc.tensor.matmul(
                                    po[:],
                                    a_sb[:, cc * 128:(cc + 1) * 128],
                                    mt[:, mi, :],
                                    start=(bkt != 0 and cc == 0),
                                    stop=(cc == nch - 1),
                                )
                            if bkt == 0:
                                nc.vector.tensor_copy(out=acc[:, blk, :],
                                                      in_=po[:])
                            elif bkt < NBUCKET - 1:
                                nc.vector.tensor_tensor(
                                    out=acc[:, blk, :], in0=acc[:, blk, :],
                                    in1=po[:], op=mybir.AluOpType.add)
                            else:
                                nc.vector.tensor_tensor(
                                    out=o_sb[:, j, :], in0=acc[:, blk, :],
                                    in1=po[:], op=mybir.AluOpType.add)
                                nc.scalar.activation(
                                    out=o_sb[:, j, :], in_=o_sb[:, j, :],
                                    func=mybir.ActivationFunctionType.Relu)
                        if bkt == NBUCKET - 1:
                            nc.sync.dma_start(
                                out=agg_out[:, g * GB:(g + 1) * GB, :],
                                in_=o_sb[:])

    nc.compile()
    return nc


def prepare_inputs(seq, edge_row, edge_col, edge_val, W, b):
    """Host-side sharding / graph partitioning. Returns (in_maps, caps)."""
    seq = np.asarray(seq, dtype=np.float32).reshape(N_NODES, IN_FT)
    r = np.asarray(edge_row).astype(np.int64)
    c = np.asarray(edge_col).astype(np.int64)
    v = np.asarray(edge_val, dtype=np.float32)
    W = np.asarray(W, dtype=np.float32).reshape(IN_FT, OUT_FT)
    b = np.asarray(b, dtype=np.float32).reshape(OUT_FT)

    # destination side
    core = r // NS
    loc = r - core * NS
    blk = loc >> 7
    rowloc = (loc & 127).astype(np.float16)
    # source side: sub-table row index
    csrc = c // NS
    crem = c % NS
    cblk = crem // 128
    cp = crem % 128
    g_of_blk = np.array([BUCKET_OF_G[blk_ // GB] for blk_ in range(NB)])
    bucket = g_of_blk[cblk]
    bnblk = np.array(BNBLK)[bucket]
    bblk0 = np.array(BBLK0)[bucket]
    lidx = (csrc * (128 * bnblk) + cp * bnblk + (cblk - bblk0)).astype(
        np.int16)

    # per-(core, block, bucket) counts -> per-(block, bucket) caps
    key = (core * NB + blk) * NBUCKET + bucket
    ngrp = NCORES * NB * NBUCKET
    counts = np.bincount(key, minlength=ngrp).reshape(NCORES, NB, NBUCKET)
    cap4 = np.maximum(1, -(-counts.max(axis=0) // 128))       # [NB, 4]
    caps_key = tuple(map(tuple, cap4.tolist()))

    startc, tot, _ = _chunk_layout(cap4)

    # slot of each edge: front-packed within its (core, block, bucket) stream
    order = np.argsort(key, kind="stable")
    key_s = key[order]
    starts = np.searchsorted(key_s, np.arange(ngrp))
    pos = np.arange(N_EDGES) - starts[key_s]
    kb = key_s % NBUCKET
    kblk = (key_s // NBUCKET) % NB
    kcore = key_s // (NBUCKET * NB)
    dest = kcore * (tot * 128) + startc[kblk, kb] * 128 + pos

    idxp = np.zeros(NCORES * tot * 128, np.int16)        # pad: row 0
    valp = np.zeros(NCORES * tot * 128, np.float16)
    rlp = np.full(NCORES * tot * 128, -1.0, np.float16)  # pad: killed
    idxp[dest] = lidx[order]
    valp[dest] = v[order].astype(np.float16)
    rlp[dest] = rowloc[order]

    idxp = idxp.reshape(NCORES, tot, 128)
    valp = valp.reshape(NCORES, tot, 128)
    rlp = rlp.reshape(NCORES, tot, 128)

    # lane-major [core, 128, tot]
    val_arr = np.ascontiguousarray(valp.transpose(0, 2, 1))
    rl_arr = np.ascontiguousarray(rlp.transpose(0, 2, 1))
    # idx 16-wrap: [core, tot*8 cols of 16] -> [core, 16, tot*8] x8 replicate
    wi = idxp.reshape(NCORES, tot * 8, 16).transpose(0, 2, 1)
    gidx_full = np.broadcast_to(wi[:, None], (NCORES, 8, 16, tot * 8))
    gidx_full = np.ascontiguousarray(
        gidx_full.reshape(NCORES, 128, tot * 8))

    biasb = np.broadcast_to((b / 128.0).astype(np.float16),
                            (128, OUT_FT)).copy()
    w3 = np.ascontiguousarray(
        W.reshape(2, 128, OUT_FT).transpose(1, 0, 2))  # [128, 2, OUT_FT]

    in_maps = []
    for k in range(NCORES):
        shard = np.zeros((NSP, IN_FT), np.float32)
        shard[:NS] = seq[k * NS:(k + 1) * NS]
        seqT_k = np.ascontiguousarray(shard.T).reshape(2, 128, NSP)
        in_maps.append({
            "seqT": seqT_k,
            "gidx": gidx_full[k],
            "val": val_arr[k],
            "rl": rl_arr[k],
            "w": w3,
            "biasb": biasb,
        })
    return in_maps, caps_key


_PROGRAMS: dict[tuple, object] = {}


def kernel(seq, edge_row, edge_col, edge_val, W, b):
    in_maps, caps = prepare_inputs(seq, edge_row, edge_col, edge_val, W, b)
    prog = _PROGRAMS.get(caps)
    if prog is None:
        prog = _PROGRAMS[caps] = build_program(caps)
    res = run_bass_kernel_spmd(prog, in_maps, core_ids=list(range(NCORES)))

    def unshard(name):
        # [128, NB, OUT_FT] partition-major -> [NS, OUT_FT] row-major
        parts = [
            res.results[k][name].transpose(1, 0, 2).reshape(NSP, OUT_FT)[:NS]
            for k in range(NCORES)
        ]
        return np.concatenate(parts)[None]

    return unshard("agg"), unshard("sf")
